# revision 3
# baseline (speedup 1.0000x reference)
"""Bass/Trainium2 kernel for a 2-layer LSTM language model.

Model: emb = inputs @ emb_w; two stacked LSTM layers (nn=1536) scanned over
T=256; logits = h1 @ out_w + out_b.

Sharding: tensor-parallel over the 4*nn gate dimension across 8 cores; core
k owns nn-slice [192k, 192k+192) of both layers (f/i/o/g columns re-packed
contiguously), keeps its c slice local, and per-step hidden chunks are
AllGathered in bf16 ([192,64] per rank -> [1536,64]).  The embedding
contribution to layer-0 gates is precomputed as one big GEMM outside the
loop: xpart = inputs @ (emb_w @ W0x) + b0.  The vocab projection is sharded
over vocab (64 cols/core) and runs inside the loop as PE filler work.

The time loop is software-pipelined one step deep so the two AllGather
streams overlap compute and each other:

- Per body i: P1 = lin0(i+1) interleaved with lin1x(i) (both unlocked by
  AG0(i)), then gates0(i+1) -> AG0(i+1) trigger, then P2 = vocab(i-1)
  interleaved with lin1h(i), gates1(i) -> AG1(i) trigger.  AG0(i+1) is
  issued BEFORE AG1(i) so the h0 recurrence (the critical cycle) never
  waits behind an AG1 window on the serial collective engine.
- The h0 reload for body i+1 is pre-issued before gates1/pack1/AG1(i) so
  it is not head-of-line blocked behind reload1(i) on the in-order DMA
  issue path.
- Layer-1 GEMMs and their whole gate chain live on PSUM/SBUF partitions
  [64:128] while layer-0/vocab use [0:64], giving the two streams disjoint
  PSUM banks.
- h packing for the AllGather: three 64x64 PE transposes (bf16) into one
  PSUM bank, one DVE copy, one DMA.
"""

import sys

sys.path.insert(0, "/opt/trn_rl_repo")

import numpy as np
import ml_dtypes

import concourse.bass as bass
import concourse.mybir as mybir
import concourse.tile as tile
from concourse import bass_utils

BF16 = mybir.dt.bfloat16
F32 = mybir.dt.float32
AF = mybir.ActivationFunctionType

T, B, V, E, NN = 256, 64, 512, 256, 1536
NCORES = 8
S = NN // NCORES          # 192  nn slice per core
G4 = 4 * S                # 768  gate cols per core
VS = V // NCORES          # 64   vocab slice per core
KT0 = NN // 128           # 12   k-tiles of h
KTE = V // 128            # 4    k-tiles of vocab (embedding GEMM contraction)

_MAXW = 1


def _split_sync_waits(nc, maxw=_MAXW):
    """walrus codegen rejects instructions with >maxw sync waits; move the
    overflow onto no-ops inserted just before (same engine, program order)."""
    for bb in nc.main_func.blocks:
        insts = bb.instructions
        i = 0
        while i < len(insts):
            inst = insts[i]
            si = inst.sync_info
            if si is not None and len(si.on_wait) > maxw:
                waits = list(si.on_wait)
                overflow, keep = waits[:-maxw], waits[-maxw:]
                inst.sync_info = mybir.SyncInfo(
                    on_wait=keep, on_update=list(si.on_update)
                )
                pos = i
                for j in range(0, len(overflow), maxw):
                    nop = mybir.InstNoOp(
                        name=nc.get_next_instruction_name(), ins=[], outs=[]
                    )
                    nop.engine = inst.engine
                    nop.sync_info = mybir.SyncInfo(
                        on_wait=overflow[j : j + maxw], on_update=[]
                    )
                    nc.register_instruction(nop, overwrite=True)
                    insts.insert(pos, nop)
                    pos += 1
                    i += 1
            i += 1


def build_program(t_steps=T, use_coll=True):
    ntok = t_steps * B
    mtok = ntok // 128

    nc = bass.Bass(
        "TRN2", target_bir_lowering=False, debug=False, num_devices=NCORES
    )

    # ---- kernel I/O (per core) ----
    inputsT = nc.dram_tensor("inputsT", [V, ntok], BF16, kind="ExternalInput").ap()
    emb_wT = nc.dram_tensor("emb_wT", [E, V], BF16, kind="ExternalInput").ap()
    w0x = nc.dram_tensor("w0x", [E, G4], BF16, kind="ExternalInput").ap()
    w0h = nc.dram_tensor("w0h", [NN, G4], BF16, kind="ExternalInput").ap()
    w1x = nc.dram_tensor("w1x", [NN, G4], BF16, kind="ExternalInput").ap()
    w1h = nc.dram_tensor("w1h", [NN, G4], BF16, kind="ExternalInput").ap()
    outw = nc.dram_tensor("outw", [NN, VS], BF16, kind="ExternalInput").ap()
    b0rep = nc.dram_tensor("b0rep", [128, G4], F32, kind="ExternalInput").ap()
    b1rep = nc.dram_tensor("b1rep", [64, G4], BF16, kind="ExternalInput").ap()
    outbrep = nc.dram_tensor("outbrep", [64, VS], F32, kind="ExternalInput").ap()
    h0T0 = nc.dram_tensor("h0T0", [NN, B], BF16, kind="ExternalInput").ap()
    h1T0 = nc.dram_tensor("h1T0", [NN, B], BF16, kind="ExternalInput").ap()
    c0in = nc.dram_tensor("c0in", [B, S], F32, kind="ExternalInput").ap()
    c1in = nc.dram_tensor("c1in", [B, S], F32, kind="ExternalInput").ap()
    identb = nc.dram_tensor("identb", [64, 64], BF16, kind="ExternalInput").ap()
    identb_hi = nc.dram_tensor("identb_hi", [128, 64], BF16, kind="ExternalInput").ap()

    logits = nc.dram_tensor(
        "logits", [t_steps, B, VS], F32, kind="ExternalOutput"
    ).ap()

    rg = [list(range(NCORES))]

    with tile.TileContext(nc) as tc:
        with (
            tc.tile_pool(name="const", bufs=1) as const,
            tc.tile_pool(name="dram", bufs=1, space="DRAM") as dram,
            tc.tile_pool(name="ring", bufs=3, space="DRAM") as ring,
            tc.tile_pool(name="sb", bufs=3) as sb,
            tc.tile_pool(name="xqp", bufs=3) as xqp,
        ):
            # ---- persistent SBUF: weights, biases, state ----
            w0h_sb = const.tile([128, KT0 * G4], BF16, tag="w0h")
            nc.sync.dma_start(
                w0h_sb[:].rearrange("p (j n) -> p j n", j=KT0),
                w0h.rearrange("(j p) n -> p j n", p=128),
            )
            w1h_sb = const.tile([128, KT0 * G4], BF16, tag="w1h")
            nc.sync.dma_start(
                w1h_sb[:].rearrange("p (j n) -> p j n", j=KT0),
                w1h.rearrange("(j p) n -> p j n", p=128),
            )
            w1x_sb = const.tile([128, KT0 * G4], BF16, tag="w1x")
            nc.sync.dma_start(
                w1x_sb[:].rearrange("p (j n) -> p j n", j=KT0),
                w1x.rearrange("(j p) n -> p j n", p=128),
            )
            outw_sb = const.tile([128, KT0 * VS], BF16, tag="outw")
            nc.sync.dma_start(
                outw_sb[:].rearrange("p (j n) -> p j n", j=KT0),
                outw.rearrange("(j p) n -> p j n", p=128),
            )
            b0rep_sb = const.tile([128, G4], F32, tag="b0rep")
            nc.sync.dma_start(b0rep_sb[:], b0rep[:])
            b1rep_sb = const.tile([64, G4], BF16, tag="b1rep")
            nc.sync.dma_start(b1rep_sb[:], b1rep[:])
            outbrep_sb = const.tile([64, VS], F32, tag="outbrep")
            nc.sync.dma_start(outbrep_sb[:], outbrep[:])
            identb_sb = const.tile([64, 64], BF16, tag="identb")
            nc.sync.dma_start(identb_sb[:], identb[:])
            identb_hi_sb = const.tile([128, 64], BF16, tag="identbh")
            nc.sync.dma_start(identb_hi_sb[:], identb_hi[:])
            c0_sb = const.tile([64, S], F32, tag="c0")
            nc.sync.dma_start(c0_sb[:], c0in[:])
            c1_sb = const.tile([128, S], F32, tag="c1")
            nc.sync.dma_start(c1_sb[64:128, :], c1in[:])

            # ---- phase 0: P = emb_w @ w0x  -> pT_sb [128, 4*G4] bf16 ----
            pT_sb = const.tile([128, KTE * G4], BF16, tag="pT")
            with (
                tc.tile_pool(name="ph0sb", bufs=2) as ph0sb,
                tc.tile_pool(name="ph0ps", bufs=2, space="PSUM") as ph0ps,
            ):
                for m in range(KTE):
                    lt = ph0sb.tile([128, 2 * 128], BF16, tag="ph0l")
                    nc.sync.dma_start(
                        lt[:].rearrange("p (k q) -> p k q", k=2),
                        emb_wT[:, 128 * m : 128 * (m + 1)].rearrange(
                            "(k p) q -> p k q", p=128
                        ),
                    )
                    rt = ph0sb.tile([128, 2 * G4], BF16, tag="ph0r")
                    nc.sync.dma_start(
                        rt[:].rearrange("p (k n) -> p k n", k=2),
                        w0x.rearrange("(k p) n -> p k n", p=128),
                    )
                    pp = ph0ps.tile([128, G4], F32, tag="ph0ps")
                    for k in range(2):
                        for n0, n1 in ((0, 512), (512, G4)):
                            nc.tensor.matmul(
                                pp[:, n0:n1],
                                lt[:, 128 * k : 128 * (k + 1)],
                                rt[:, G4 * k + n0 : G4 * k + n1],
                                start=(k == 0),
                                stop=(k == 1),
                            )
                    nc.vector.tensor_copy(pT_sb[:, G4 * m : G4 * (m + 1)], pp[:])

            # ---- phase 1: xpart = inputsT.T @ P + b0  -> DRAM [ntok, G4] bf16
            xpart = dram.tile([ntok, G4], BF16, tag="xpart")
            with (
                tc.tile_pool(name="ph1sb", bufs=3) as ph1sb,
                tc.tile_pool(name="ph1ps", bufs=2, space="PSUM") as ph1ps,
            ):
                for m in range(mtok):
                    lt = ph1sb.tile([128, KTE * 128], BF16, tag="ph1l")
                    nc.sync.dma_start(
                        lt[:].rearrange("p (k q) -> p k q", k=KTE),
                        inputsT[:, 128 * m : 128 * (m + 1)].rearrange(
                            "(k p) q -> p k q", p=128
                        ),
                    )
                    pp = ph1ps.tile([128, G4], F32, tag="ph1ps")
                    for k in range(KTE):
                        for n0, n1 in ((0, 512), (512, G4)):
                            nc.tensor.matmul(
                                pp[:, n0:n1],
                                lt[:, 128 * k : 128 * (k + 1)],
                                pT_sb[:, G4 * k + n0 : G4 * k + n1],
                                start=(k == 0),
                                stop=(k == KTE - 1),
                            )
                    xf = ph1sb.tile([128, G4], BF16, tag="ph1o")
                    nc.vector.tensor_add(xf[:], pp[:], b0rep_sb[:])
                    nc.sync.dma_start(xpart[128 * m : 128 * (m + 1), :], xf[:])

            # ---- initial hidden state tiles (feature-major [128, 12*64]) ----
            def load_hT(src, tag):
                """DRAM [NN, B] -> SBUF [128, KT0*B], split in two DMAs so the
                first k-tiles are available earlier."""
                t = sb.tile([128, KT0 * B], BF16, tag=tag)
                half = KT0 // 2
                for u in range(2):
                    nc.sync.dma_start(
                        t[:, half * B * u : half * B * (u + 1)].rearrange(
                            "p (j b) -> p j b", j=half
                        ),
                        src[768 * u : 768 * (u + 1), :].rearrange(
                            "(j p) b -> p j b", p=128
                        ),
                    )
                return t

            h0_prev = load_hT(h0T0, "h0")
            h1_prev = load_hT(h1T0, "h1")

            with (
                tc.tile_pool(name="ps0p", bufs=1, space="PSUM") as ps0p,
                tc.tile_pool(name="ps1p", bufs=2, space="PSUM") as ps1p,
                tc.tile_pool(name="psVp", bufs=1, space="PSUM") as psVp,
                tc.tile_pool(name="psTp", bufs=1, space="PSUM") as psTp,
            ):
                def load_xq(t):
                    xq = xqp.tile([B, G4], BF16, tag="xq")
                    nc.sync.dma_start(xq[:], xpart[B * t : B * (t + 1), :])
                    return xq

                def gate_chain(ps, c_sb, layer):
                    """sigmoid/tanh gates + c update -> h_new bf16 [64, S].
                    layer 0 at partitions [0:64], layer 1 at [64:128]."""
                    lo, hi = (0, 64) if layer == 0 else (64, 128)
                    act = sb.tile([hi, G4], F32, tag=f"act{layer}")
                    a = act[lo:hi, :]
                    c = c_sb[lo:hi, :] if layer == 1 else c_sb[:]
                    p = ps[lo:hi, :] if layer == 1 else ps[:]
                    nc.scalar.activation(a[:, 0 : 3 * S], p[:, 0 : 3 * S], AF.Sigmoid)
                    nc.scalar.activation(a[:, 3 * S : G4], p[:, 3 * S : G4], AF.Tanh)
                    fc = sb.tile([hi, S], F32, tag=f"fc{layer}")
                    nc.vector.tensor_mul(fc[lo:hi, :], a[:, 0:S], c)
                    ig = sb.tile([hi, S], F32, tag=f"ig{layer}")
                    nc.vector.tensor_mul(
                        ig[lo:hi, :], a[:, S : 2 * S], a[:, 3 * S : G4]
                    )
                    nc.vector.tensor_add(c, fc[lo:hi, :], ig[lo:hi, :])
                    th = sb.tile([hi, S], F32, tag=f"th{layer}")
                    nc.scalar.activation(th[lo:hi, :], c, AF.Tanh)
                    hn = sb.tile([hi, S], BF16, tag=f"hn{layer}")
                    nc.vector.tensor_mul(hn[lo:hi, :], a[:, 2 * S : 3 * S], th[lo:hi, :])
                    return hn

                def pack_out(hn, layer, tag):
                    """h [64, S] (base 0 or 64) -> ag_in DRAM [S, 64] via three
                    64x64 PE transposes + one DVE copy + one DMA."""
                    lo, hi = (0, 64) if layer == 0 else (64, 128)
                    idn = identb_sb[:] if layer == 0 else identb_hi_sb[64:128, :]
                    tp = psTp.tile([64, 3 * B], BF16, tag="tp")
                    for w in range(3):
                        nc.tensor.matmul(
                            tp[:, B * w : B * (w + 1)],
                            hn[lo:hi, 64 * w : 64 * (w + 1)],
                            idn,
                            is_transpose=True,
                        )
                    tb = sb.tile([64, 3 * B], BF16, tag=f"tb{layer}")
                    nc.vector.tensor_copy(tb[:], tp[:])
                    ag_in = ring.tile([S, B], BF16, tag=tag)
                    nc.sync.dma_start(
                        ag_in[:].rearrange("(w p) b -> p w b", p=64),
                        tb[:].rearrange("p (w b) -> p w b", w=3),
                    )
                    return ag_in

                def all_gather(ag_in, tag):
                    ag_out = ring.tile([NN, B], BF16, tag=tag)
                    if use_coll:
                        nc.gpsimd.collective_compute(
                            "AllGather", mybir.AluOpType.bypass,
                            replica_groups=rg,
                            ins=[ag_in.opt()], outs=[ag_out.opt()],
                        )
                    else:
                        nc.sync.dma_start(ag_out[0:S, :], ag_in[:])
                    return ag_out

                def vocab_proj(h_tile, t_out):
                    lo = psVp.tile([B, VS], F32, tag="lout")
                    for j in range(KT0):
                        nc.tensor.matmul(
                            lo[:],
                            h_tile[:, B * j : B * (j + 1)],
                            outw_sb[:, VS * j : VS * (j + 1)],
                            start=(j == 0),
                            stop=(j == KT0 - 1),
                        )
                    lsb = sb.tile([B, VS], F32, tag="lsb")
                    nc.vector.tensor_add(lsb[:], lo[:], outbrep_sb[:])
                    nc.sync.dma_start(logits[t_out], lsb[:])

                def lin0_mms(ps0, xq, h_tile):
                    """lin0 matmuls (low half): ident-xq first, then 12 k-tiles,
                    emitted as a list of thunks for interleaving."""
                    mms = []
                    for n0, n1 in ((0, 512), (512, G4)):
                        mms.append(lambda n0=n0, n1=n1: nc.tensor.matmul(
                            ps0[:, n0:n1], identb_sb[:], xq[:, n0:n1],
                            start=True, stop=False,
                        ))
                        for j in range(KT0):
                            mms.append(lambda n0=n0, n1=n1, j=j: nc.tensor.matmul(
                                ps0[:, n0:n1],
                                h_tile[:, B * j : B * (j + 1)],
                                w0h_sb[:, G4 * j + n0 : G4 * j + n1],
                                start=False, stop=(j == KT0 - 1),
                            ))
                    return mms

                def lin1_mms(ps1, h_tile, w_sb, first, last, bias=None):
                    """lin1 matmuls (high half, psum partitions [64:128])."""
                    mms = []
                    for n0, n1 in ((0, 512), (512, G4)):
                        if first:
                            mms.append(lambda n0=n0, n1=n1: nc.tensor.matmul(
                                ps1[64:128, n0:n1], identb_sb[:], b1rep_sb[:, n0:n1],
                                start=True, stop=False,
                            ))
                        for j in range(KT0):
                            mms.append(lambda n0=n0, n1=n1, j=j: nc.tensor.matmul(
                                ps1[64:128, n0:n1],
                                h_tile[:, B * j : B * (j + 1)],
                                w_sb[:, G4 * j + n0 : G4 * j + n1],
                                start=False, stop=(last and j == KT0 - 1),
                            ))
                    return mms

                def interleave(a, b):
                    out = []
                    na, nb = len(a), len(b)
                    n = max(na, nb)
                    for i in range(n):
                        if i < na:
                            out.append(a[i])
                        if i < nb:
                            out.append(b[i])
                    for f in out:
                        f()

                # ---- prologue: step 0 of layer 0 ----
                xq_cur = load_xq(0)
                xq_next = load_xq(1) if t_steps > 1 else None
                ps0 = ps0p.tile([64, G4], F32, tag="lin0")
                interleave(lin0_mms(ps0, xq_cur, h0_prev), [])
                h0n = gate_chain(ps0, c0_sb, 0)
                ag0_in = pack_out(h0n, 0, "ag0i")
                ag0_out = all_gather(ag0_in, "ag0o")

                lout_prev = None  # h1 tile pending vocab projection
                h0_cur = None

                for i in range(t_steps):
                    # reload h0_full(i) — for i>=1 the DMA was pre-issued in
                    # body i-1 so it is not queued behind reload1(i-1)
                    if i == 0:
                        h0_cur = load_hT(ag0_out[:], "h0")

                    # P1: lin0(i+1) [low] || lin1x(i) [high]
                    ps1 = ps1p.tile([128, G4], F32, tag="lin1")
                    mm_hi = lin1_mms(ps1, h0_cur, w1x_sb, first=True, last=False)
                    if i + 1 < t_steps:
                        ps0 = ps0p.tile([64, G4], F32, tag="lin0")
                        mm_lo = lin0_mms(ps0, xq_next, h0_cur)
                    else:
                        mm_lo = []
                    interleave(mm_lo, mm_hi)

                    # prefetch xpart for step i+2
                    if i + 2 < t_steps:
                        xq_next2 = load_xq(i + 2)
                    else:
                        xq_next2 = None

                    # gates0(i+1) -> AG0(i+1) trigger (before AG1(i)!)
                    if i + 1 < t_steps:
                        h0n = gate_chain(ps0, c0_sb, 0)
                        ag0_in = pack_out(h0n, 0, "ag0i")
                        ag0_out = all_gather(ag0_in, "ag0o")

                    # P2: vocab(i-1) [low] || lin1h(i) [high]
                    mm_hi = lin1_mms(ps1, h1_prev, w1h_sb, first=False, last=True)
                    if lout_prev is not None:
                        mm_lo = []
                        lo = psVp.tile([B, VS], F32, tag="lout")
                        for j in range(KT0):
                            mm_lo.append(lambda j=j, lo=lo, h=lout_prev: nc.tensor.matmul(
                                lo[:],
                                h[:, B * j : B * (j + 1)],
                                outw_sb[:, VS * j : VS * (j + 1)],
                                start=(j == 0), stop=(j == KT0 - 1),
                            ))
                        interleave(mm_lo, mm_hi)
                        lsb = sb.tile([B, VS], F32, tag="lsb")
                        nc.vector.tensor_add(lsb[:], lo[:], outbrep_sb[:])
                        nc.sync.dma_start(logits[i - 1], lsb[:])
                    else:
                        interleave([], mm_hi)

                    # pre-issue next body's h0 reload ahead of reload1(i)
                    if i + 1 < t_steps:
                        h0_next = load_hT(ag0_out[:], "h0")
                    else:
                        h0_next = None

                    # gates1(i) -> AG1(i)
                    h1n = gate_chain(ps1, c1_sb, 1)
                    ag1_in = pack_out(h1n, 1, "ag1i")
                    ag1_out = all_gather(ag1_in, "ag1o")
                    h1_cur = load_hT(ag1_out[:], "h1")

                    h0_prev, h1_prev, lout_prev = h0_cur, h1_cur, h1_cur
                    h0_cur = h0_next
                    xq_cur, xq_next = xq_next, xq_next2

                # tail: vocab projection for the last step
                vocab_proj(lout_prev, t_steps - 1)

    _split_sync_waits(nc)
    return nc


_PROGRAM_CACHE = {}


def _get_program(t_steps=T, use_coll=True):
    key = (t_steps, use_coll)
    if key not in _PROGRAM_CACHE:
        _PROGRAM_CACHE[key] = build_program(t_steps, use_coll)
    return _PROGRAM_CACHE[key]


def make_in_maps(inputs, emb_w, lstm_w0, lstm_b0, lstm_w1, lstm_b1,
                 out_w, out_b, h0, c0, h1, c1, t_steps=T):
    bf16 = ml_dtypes.bfloat16
    f32 = np.float32
    ntok = t_steps * B
    inputsT = np.ascontiguousarray(inputs.reshape(ntok, V).T).astype(bf16)
    emb_wT = np.ascontiguousarray(emb_w.T).astype(bf16)
    identb = np.eye(64).astype(bf16)
    identb_hi = np.zeros((128, 64), np.float32)
    identb_hi[64:128, :] = np.eye(64)
    identb_hi = identb_hi.astype(bf16)

    def gate_cols(w, k):
        # [in, 4*NN] -> per-core [in, 4*S] with [f|i|o|g] blocks
        return np.concatenate(
            [w[:, g * NN + k * S : g * NN + (k + 1) * S] for g in range(4)],
            axis=1,
        )

    in_maps = []
    for k in range(NCORES):
        w0k = gate_cols(lstm_w0, k)
        w1k = gate_cols(lstm_w1, k)
        b0k = gate_cols(lstm_b0[None, :], k)[0]
        b1k = gate_cols(lstm_b1[None, :], k)[0]
        in_maps.append({
            "inputsT": inputsT,
            "emb_wT": emb_wT,
            "w0x": np.ascontiguousarray(w0k[:E]).astype(bf16),
            "w0h": np.ascontiguousarray(w0k[E:]).astype(bf16),
            "w1x": np.ascontiguousarray(w1k[:NN]).astype(bf16),
            "w1h": np.ascontiguousarray(w1k[NN:]).astype(bf16),
            "outw": np.ascontiguousarray(
                out_w[:, k * VS : (k + 1) * VS]
            ).astype(bf16),
            "b0rep": np.broadcast_to(b0k.astype(f32), (128, G4)).copy(),
            "b1rep": np.broadcast_to(b1k, (64, G4)).astype(bf16).copy(),
            "outbrep": np.broadcast_to(
                out_b[k * VS : (k + 1) * VS].astype(f32), (64, VS)
            ).copy(),
            "h0T0": np.ascontiguousarray(h0.T).astype(bf16),
            "h1T0": np.ascontiguousarray(h1.T).astype(bf16),
            "c0in": np.ascontiguousarray(c0[:, k * S : (k + 1) * S]).astype(f32),
            "c1in": np.ascontiguousarray(c1[:, k * S : (k + 1) * S]).astype(f32),
            "identb": identb,
            "identb_hi": identb_hi,
        })
    return in_maps


def kernel(inputs, emb_w, lstm_w0, lstm_b0, lstm_w1, lstm_b1,
           out_w, out_b, h0, c0, h1, c1, _trace=False):
    inputs = np.asarray(inputs, dtype=np.float32)
    t_steps = inputs.shape[0]
    nc = _get_program(t_steps)
    in_maps = make_in_maps(
        inputs,
        np.asarray(emb_w, np.float32), np.asarray(lstm_w0, np.float32),
        np.asarray(lstm_b0, np.float32), np.asarray(lstm_w1, np.float32),
        np.asarray(lstm_b1, np.float32), np.asarray(out_w, np.float32),
        np.asarray(out_b, np.float32), np.asarray(h0, np.float32),
        np.asarray(c0, np.float32), np.asarray(h1, np.float32),
        np.asarray(c1, np.float32), t_steps=t_steps,
    )
    res = bass_utils.run_bass_kernel_spmd(
        nc, in_maps, core_ids=list(range(NCORES)), trace=_trace
    )
    out = np.concatenate(
        [res.results[k]["logits"] for k in range(NCORES)], axis=2
    )
    if _trace:
        kernel.last_results = res
    return out.astype(np.float32)
